# revision 69
# baseline (speedup 1.0000x reference)
"""Channel-attention (XCA) block on 8 trn2 NeuronCores, data-parallel over batch.

Per core: x (4096, 768) -> qkv -> per-head channel attention (96x96 scores over
l2-normalized q,k transposed to (Ch, N)) -> proj.  All big matmuls run in bf16
with fp32 PSUM accumulation; norms/softmax in fp32.

v16: single fused pass over tokens. Per 128-token block: all-head q|k qkv
matmuls (3 x N=512), incremental score accumulation for all 8 heads into two
held-open PSUM banks, squares on ACT from PSUM, running sum-of-squares add on
the Pool engine. q|k activations only live in a 4-block rotating buffer.
The whole output path stays folded into one GEMM: y = x @ W2 + b with
W2 = Wv . Q, Q = blockdiag(attn_h^T) . Wproj, built in a single softmax tail
whose DVE/ACT chains hide under the WvT PE transposes.  Softmax/norm use a
single ln/exp activation table (no table reloads): 1/max(sqrt(v), eps) =
exp(-0.5 ln(max(v, eps^2))).
"""

import numpy as np
from contextlib import ExitStack

import bass_rust
import concourse.bass as bass
import concourse.tile as tile
from concourse import mybir
from concourse.masks import make_identity
from concourse.bass_utils import run_bass_kernel_spmd

F32 = mybir.dt.float32
BF = mybir.dt.bfloat16
AF = mybir.ActivationFunctionType

P = 128          # partitions
N = 4096         # tokens per core (batch element)
C = 768          # channels
H = 8            # heads
CH = 96          # channels per head
KC = C // P      # 6 contraction chunks of 128
NB = N // P      # 32 token blocks of 128
QK = 2 * C       # q|k columns per token block
NCH = 3          # qkv PSUM chunks of 512 per block
EPS2 = 1e-24     # eps^2 clamp on sum-of-squares (torch F.normalize eps=1e-12)
LAG = 13         # qkv blocks trail the transpose loop by this many blocks
ROT = 3          # rotating q|k block buffers


def build_nc():
    nc = bass.Bass()

    x_d = nc.dram_tensor("x", [N, C], F32, kind="ExternalInput")
    wqkv_d = nc.dram_tensor("Wqkv", [C, 3 * C], F32, kind="ExternalInput")
    temp_d = nc.dram_tensor("temperature", [H], F32, kind="ExternalInput")
    wproj_d = nc.dram_tensor("Wproj", [C, C], F32, kind="ExternalInput")
    bproj_d = nc.dram_tensor("bproj", [C], F32, kind="ExternalInput")
    y_d = nc.dram_tensor("y", [N, C], F32, kind="ExternalOutput")

    with ExitStack() as ctx:
        tc = ctx.enter_context(tile.TileContext(nc))
        persist = ctx.enter_context(tc.tile_pool(name="persist", bufs=1))

        # persistent SBUF: xT[c%128, c//128, n] = x[n, c]  (bf16)
        xT = persist.tile([P, KC, N], BF)
        # Wv^T per head/chunk: wvT[d, kc, h, j] = Wqkv[kc*128+j, 2C + h*96 + d]
        wvT = persist.tile([CH, KC, H, P], BF)
        # Wproj rows by head: wp96[c, h, jo] = Wproj[h*96 + c, jo]
        wp96 = persist.tile([CH, H, C], BF)
        # Q[d, h, jo] = sum_c attn_h[c, d] Wproj[h*96+c, jo]
        q_sb = persist.tile([CH, H, C], BF)
        bias_sb = persist.tile([P, C], F32)

        ident128b = persist.tile([P, P], BF)
        make_identity(nc, ident128b)
        ident128f = persist.tile([P, P], F32)
        make_identity(nc, ident128f)
        ones_colf = persist.tile([P, 1], F32)    # norm-matmul lhsT (K=128, M=1)
        nc.vector.memset(ones_colf, 1.0)
        ones_row = persist.tile([1, P], BF)      # bias-matmul lhsT (K=1, M=128)
        nc.vector.memset(ones_row, 1.0)
        one1 = persist.tile([1, 1], F32)         # row->col matmul rhs
        nc.vector.memset(one1, 1.0)
        ones96 = persist.tile([1, CH], F32)
        nc.vector.memset(ones96, 1.0)

        temp_sb = persist.tile([1, H], F32)
        bstage = persist.tile([1, C], F32)
        bstage_bf = persist.tile([1, C], BF)

        # stream-phase pools on the RIGHT allocation stack (released while
        # the left-side tail pools stay open; release is LIFO per side)
        qkctx = ctx.enter_context(ExitStack())
        # wqk[c%128, c//128, j] = Wqkv[c, j] for the q|k columns j in [0, 2C)
        wqk_pool = qkctx.enter_context(tc.tile_pool(name="wqk", bufs=1, side="right"))
        wqk = wqk_pool.tile([P, KC, QK], BF)
        qkr_pool = qkctx.enter_context(tc.tile_pool(name="qkr", bufs=ROT, side="right"))
        naccpool = qkctx.enter_context(tc.tile_pool(name="nacc", bufs=1, side="right"))
        nacc = naccpool.tile([P, QK], F32)
        # v-column staging lives here (right stack) so the tail's WvT
        # transposes can still read it after the stream's stage pools close
        wvstage = qkctx.enter_context(tc.tile_pool(name="wvstage", bufs=6,
                                                   side="right"))
        # qkv PSUM rotation on its own stack (top of the right side) so its
        # four banks release to the tail pools right after the stream
        qkpsctx = ctx.enter_context(ExitStack())
        qkps = qkpsctx.enter_context(tc.tile_pool(name="qkps", bufs=4, space="PSUM",
                                                  side="right"))

        softctx = ctx.enter_context(ExitStack())
        small = softctx.enter_context(tc.tile_pool(name="small", bufs=2))
        # all-head scores, held open across the whole stream: head h lives in
        # bank h//4 at slice [:, h//4, h%4, 0:96] (pad to 128 keeps each bank's
        # four heads exactly filling its 2KB)
        sps = softctx.enter_context(tc.tile_pool(name="sps", bufs=1, space="PSUM"))
        s_all = sps.tile([CH, 2, 4, P], F32)

        # deferred ops (Wproj loads, bias build) paced one per token block
        deferred = []

        def emit_deferred(k=1):
            for _ in range(k):
                if deferred:
                    deferred.pop(0)()

        rot = [None] * ROT   # rotating q|k bf16 block tiles
        pend = {"nb": None}

        def scores_block(nb):
            qkb = rot[nb % ROT]
            for h in range(H):
                nc.tensor.matmul(
                    s_all[:, h // 4, h % 4, 0:CH],
                    qkb[:, h * CH:(h + 1) * CH],
                    qkb[:, C + h * CH: C + (h + 1) * CH],
                    start=(nb == 0 and h % 4 == 0),
                    stop=(nb == NB - 1 and h % 4 == 3))

        def qkv_block(nb):
            """all-head q|k matmuls for one token block (3 PSUM chunks of
            512), eviction into the rotating buffer, squares on ACT from
            PSUM, Pool running sum; scores for the previous block lead (their
            operands' evictions are a block old, so the PE never waits)."""
            if pend["nb"] is not None:
                scores_block(pend["nb"])
            qkb = qkr_pool.tile([P, QK], BF, tag="qkr")
            rot[nb % ROT] = qkb
            sqt = naccpool.tile([P, NCH, 512], F32, tag="sqt", bufs=2)
            for chunk in range(NCH):
                qkp = qkps.tile([P, 512], F32, tag="qkp")
                for kc in range(KC):
                    nc.tensor.matmul(
                        qkp, xT[:, kc, nb * P:(nb + 1) * P],
                        wqk[:, kc, chunk * 512:(chunk + 1) * 512],
                        start=(kc == 0), stop=(kc == KC - 1))
                nc.vector.tensor_copy(qkb[:, chunk * 512:(chunk + 1) * 512], qkp)
                nc.scalar.activation(sqt[:, chunk, :], qkp, AF.Square)
            emit_deferred()
            if nb == 0:
                nc.gpsimd.tensor_copy(nacc, sqt)
            else:
                nc.gpsimd.tensor_add(nacc, nacc, sqt)
            pend["nb"] = nb

        def load_wp(h):
            st = wpstage.tile([CH, C], F32, tag="wpst")
            nc.sync.dma_start(out=st, in_=wproj_d[h * CH:(h + 1) * CH, :])
            nc.vector.tensor_copy(wp96[:, h, :], st)

        def build_bias(half):
            a, b = (0, 384) if half == 0 else (384, C)
            bps = tinyps.tile([P, 384], F32, tag="tp")
            nc.tensor.matmul(bps, ones_row, bstage_bf[0:1, a:b],
                             start=True, stop=True)
            nc.vector.tensor_copy(bias_sb[:, a:b], bps)

        wpstage = softctx.enter_context(tc.tile_pool(name="wpstage", bufs=2))

        for h in range(H):
            deferred.append(lambda h=h: load_wp(h))

        # ---- fused stream: x -> xT (bf16 transposes) + Wqkv load + all-head
        # qkv/scores/norm blocks.  x rides the sync ring; Wqkv the Activation
        # ring (q|k chunks first, v chunks later for the tail's WvT build).
        sv_tiles = []
        with tc.tile_pool(name="xstage", bufs=2) as xstage, \
             tc.tile_pool(name="xbstage", bufs=2) as xbstage, \
             tc.tile_pool(name="wstage", bufs=1) as wstage, \
             tc.tile_pool(name="tps", bufs=2, space="PSUM") as tps:
            for nb in range(NB + LAG):
                if nb < NB:
                    xt_ = xstage.tile([P, C], F32, tag="x")
                    nc.sync.dma_start(out=xt_, in_=x_d[nb * P:(nb + 1) * P, :])
                    xb = xbstage.tile([P, C], BF, tag="xb")
                    nc.scalar.activation(xb, xt_, AF.Copy)
                    tall = tps.tile([P, KC, P], BF, tag="t")
                    for kc in range(KC):
                        nc.tensor.matmul(tall[:, kc, :], xb[:, kc * P:(kc + 1) * P],
                                         ident128b, is_transpose=True,
                                         start=(kc == 0), stop=(kc == KC - 1))
                    nc.vector.tensor_copy(xT[:, :, nb * P:(nb + 1) * P], tall)
                if 1 <= nb <= 2 * KC:
                    # half-chunks of the q|k columns (keeps the stage small)
                    kc, hf = (nb - 1) // 2, (nb - 1) % 2
                    st = wstage.tile([P, C], F32, tag="wst")
                    nc.scalar.dma_start(
                        out=st, in_=wqkv_d[kc * P:(kc + 1) * P, hf * C:(hf + 1) * C])
                    nc.vector.tensor_copy(wqk[:, kc, hf * C:(hf + 1) * C], st)
                if 13 <= nb <= 18:
                    kc = nb - 13
                    sv = wvstage.tile([P, H, CH], F32, tag="wsv")
                    nc.scalar.dma_start(out=sv,
                                        in_=wqkv_d[kc * P:(kc + 1) * P, 2 * C:3 * C])
                    sv_tiles.append(sv)
                if nb == KC + 1:
                    nc.sync.dma_start(out=temp_sb,
                                      in_=temp_d.rearrange("(a h) -> a h", a=1))
                    nc.sync.dma_start(out=bstage,
                                      in_=bproj_d.rearrange("(a c) -> a c", a=1))
                    nc.vector.tensor_copy(bstage_bf, bstage)
                if nb >= LAG:
                    qkv_block(nb - LAG)

        # ---- tail: flush last scores, WvT transposes (PE work that hides
        # the Pool sum drain + the softmax chains), norm row, per-head
        # softmax + Q builds, then W2 and the output GEMM.
        scores_block(pend["nb"])
        qkpsctx.close()
        wvtps = softctx.enter_context(tc.tile_pool(name="wvtps", bufs=1, space="PSUM"))
        tinyps = softctx.enter_context(tc.tile_pool(name="tinyps", bufs=1, space="PSUM"))
        qps = softctx.enter_context(tc.tile_pool(name="qps", bufs=2, space="PSUM"))

        for kc in range(KC):
            for hw in range(2):
                wvtp = wvtps.tile([CH, 4, P], F32, tag="wvt")
                for hh in range(4):
                    nc.tensor.matmul(
                        wvtp[:, hh, :],
                        sv_tiles[kc][:, hw * 4 + hh, :],
                        ident128f, is_transpose=True,
                        start=(hh == 0), stop=(hh == 3))
                nc.vector.tensor_copy(wvT[:, kc, hw * 4:(hw + 1) * 4, :], wvtp)

        # norm row for all heads: nqk chunks of 512 (PSUM bank-sized), then
        # rqk = 1/max(sqrt(v), eps) = exp(-0.5 ln(max(v, eps^2)))
        rqk = small.tile([1, QK], F32, tag="rqk")
        for chunk in range(NCH):
            nqk = tinyps.tile([1, 512], F32, tag="tp")
            nc.tensor.matmul(nqk, ones_colf, nacc[:, chunk * 512:(chunk + 1) * 512],
                             start=True, stop=True)
            vv = small.tile([1, 512], F32, tag="vv")
            nc.vector.tensor_scalar_max(vv, nqk, EPS2)
            lnv = small.tile([1, 512], F32, tag="lnv")
            nc.scalar.activation(lnv, vv, AF.Ln)
            nc.scalar.activation(rqk[0:1, chunk * 512:(chunk + 1) * 512], lnv,
                                 AF.Exp, scale=-0.5)

        deferred.append(lambda: build_bias(0))
        deferred.append(lambda: build_bias(1))
        emit_deferred(len(deferred))

        # per-head norm-derived tiles (tiny PE matmuls); the DVE/ACT softmax
        # chains for all heads then drain while the PE moves on
        rq_cols, r_sbs = [], []
        for h in range(H):
            rq_ps = tinyps.tile([CH, 1], F32, tag="tp")
            nc.tensor.matmul(rq_ps, rqk[0:1, h * CH:(h + 1) * CH], one1,
                             start=True, stop=True)
            rq_col = small.tile([CH, 1], F32, tag="rqc", bufs=H)
            nc.vector.tensor_copy(rq_col, rq_ps)
            tempb = small.tile([1, CH], F32, tag="tb")
            nc.scalar.activation(tempb, ones96, AF.Copy,
                                 scale=temp_sb[0:1, h:h + 1])
            r_ps = tinyps.tile([CH, CH], F32, tag="tp")
            nc.tensor.matmul(r_ps, tempb,
                             rqk[0:1, C + h * CH: C + (h + 1) * CH],
                             start=True, stop=True)
            r_sb = small.tile([CH, CH], F32, tag="rsb", bufs=H)
            nc.vector.tensor_copy(r_sb, r_ps)
            rq_cols.append(rq_col)
            r_sbs.append(r_sb)

        attns = []
        for h in range(H):
            z_sb = small.tile([CH, CH], F32, tag="z", bufs=4)
            nc.vector.tensor_mul(z_sb, s_all[:, h // 4, h % 4, 0:CH], r_sbs[h])
            e_sb = small.tile([CH, CH], BF, tag="e", bufs=4)
            sume = small.tile([CH, 1], F32, tag="se", bufs=4)
            nc.scalar.activation(e_sb, z_sb, AF.Exp,
                                 scale=rq_cols[h], accum_out=sume)
            rden = small.tile([CH, 1], F32, tag="rd", bufs=4)
            nc.vector.reciprocal(rden, sume)
            attn_s = small.tile([CH, CH], BF, tag="at", bufs=H)
            nc.scalar.activation(attn_s, e_sb, AF.Copy, scale=rden)
            attns.append(attn_s)

        for h in range(H):
            # Q[d, h, :] = sum_c attn_h[c, d] Wproj[h*96+c, :]; attn is the
            # stationary operand directly (c on partitions) -- no transpose
            qpa = qps.tile([CH, 512], F32, tag="qpa")
            qpb = qps.tile([CH, 256], F32, tag="qpb")
            nc.tensor.matmul(qpa, attns[h], wp96[:, h, 0:512],
                             start=True, stop=True)
            nc.tensor.matmul(qpb, attns[h], wp96[:, h, 512:C],
                             start=True, stop=True)
            nc.vector.tensor_copy(q_sb[:, h, 0:512], qpa)
            nc.vector.tensor_copy(q_sb[:, h, 512:C], qpb)

        softctx.close()
        qkctx.close()

        # ---- W2 = Wv . Q (per j-chunk, accumulated over heads), then the
        # single output GEMM y = x @ W2 + bias.
        cctx = ctx.enter_context(ExitStack())
        w2_pool = cctx.enter_context(tc.tile_pool(name="w2", bufs=1, side="right"))
        w2 = w2_pool.tile([P, KC, C], BF)
        yout = cctx.enter_context(tc.tile_pool(name="yout", bufs=3, side="right"))
        w2ps = cctx.enter_context(tc.tile_pool(name="w2ps", bufs=2, space="PSUM",
                                               side="right"))
        yps = cctx.enter_context(tc.tile_pool(name="yps", bufs=2, space="PSUM",
                                              side="right"))

        for jkc in range(KC):
            w2a = w2ps.tile([P, 512], F32, tag="w2a")
            w2b = w2ps.tile([P, 256], F32, tag="w2b")
            for h in range(H):
                nc.tensor.matmul(w2a, wvT[:, jkc, h, :], q_sb[:, h, 0:512],
                                 start=(h == 0), stop=(h == H - 1))
                nc.tensor.matmul(w2b, wvT[:, jkc, h, :], q_sb[:, h, 512:C],
                                 start=(h == 0), stop=(h == H - 1))
            nc.vector.tensor_copy(w2[:, jkc, 0:512], w2a)
            nc.vector.tensor_copy(w2[:, jkc, 512:C], w2b)

        for nb in range(NB):
            y1 = yps.tile([P, 512], F32, tag="y1")
            y2 = yps.tile([P, 256], F32, tag="y2")
            for kc in range(KC):
                nc.tensor.matmul(y1, xT[:, kc, nb * P:(nb + 1) * P],
                                 w2[:, kc, 0:512],
                                 start=(kc == 0), stop=(kc == KC - 1))
            for kc in range(KC):
                nc.tensor.matmul(y2, xT[:, kc, nb * P:(nb + 1) * P],
                                 w2[:, kc, 512:C],
                                 start=(kc == 0), stop=(kc == KC - 1))
            ysb = yout.tile([P, C], F32, tag="y")
            nc.vector.tensor_add(ysb[:, 0:512], y1, bias_sb[:, 0:512])
            nc.vector.tensor_add(ysb[:, 512:C], y2, bias_sb[:, 512:C])
            nc.sync.dma_start(out=y_d[nb * P:(nb + 1) * P, :], in_=ysb)

        cctx.close()

    # Split multi-wait sync conditions into EventSemaphore instructions —
    # walrus' ACT/DVE instruction structs encode at most one wait.
    bass_rust.generate_event_semaphores(nc)
    return nc


def _in_maps(x, Wqkv, temperature, Wproj, bproj):
    x = np.asarray(x)  # plain numpy before slicing (inputs may be jax arrays)
    wqkv = np.ascontiguousarray(Wqkv, dtype=np.float32)
    temp = np.ascontiguousarray(temperature, dtype=np.float32).reshape(H)
    wproj = np.ascontiguousarray(Wproj, dtype=np.float32)
    bp = np.ascontiguousarray(bproj, dtype=np.float32)
    return [
        {"x": np.ascontiguousarray(x[b], dtype=np.float32), "Wqkv": wqkv,
         "temperature": temp, "Wproj": wproj, "bproj": bp}
        for b in range(x.shape[0])
    ]


def run(x, Wqkv, temperature, Wproj, bproj, trace=False):
    nc = build_nc()
    in_maps = _in_maps(x, Wqkv, temperature, Wproj, bproj)
    res = run_bass_kernel_spmd(nc, in_maps, core_ids=list(range(len(in_maps))),
                               trace=trace)
    out = np.stack([res.results[b]["y"] for b in range(len(in_maps))], axis=0)
    return out.astype(np.float32), res


def kernel(x, Wqkv, temperature, Wproj, bproj):
    out, _ = run(x, Wqkv, temperature, Wproj, bproj, trace=False)
    return out
